# revision 16
# baseline (speedup 1.0000x reference)
"""Trainium2 Bass kernel for nn_MultiHeadAttention_45019847196962.

Reference computation (per batch b):
    q = Q @ Wq + bq                 # (Lq, H*D)
    v = V @ Wv + bv                 # (Lk, H*D)   (used as both keys and values)
    scores = q_h @ v_h^T            # per head, no 1/sqrt(d) scale
    align  = softmax(scores, -1)
    attn   = align @ v_h            # concat heads -> (Lq, H*D)
    out    = tanh([attn | Q] @ Wf + bf)

Sharding: data-parallel over batch. 16 batches / 8 cores = 2 batches per
core; weights replicated. No collectives.

Per-core dataflow (all matmul operands bf16, fp32 PSUM accumulation):
  - Inputs are cast fp32->bf16 by DRAM->DRAM SWDGE DMAs (batch 0 in
    256-row chunks so the dependent HWDGE DMA-transposes pipeline behind
    the cast instead of waiting for the full tensor), then loaded
    transposed (Q^T, V^T: contraction dim on partitions).
  - qT = Wq^T Q^T and vT = Wv^T V^T with the bias added per-partition
    during the PSUM->SBUF copy.  vT is the only V projection; the
    natural-layout v (+ ones column for the softmax denominator) is
    produced by 32 [128,128] SBUF->SBUF DMA transposes of vT per batch,
    which keeps the duplicate projection off the Tensor engine.
  - scores^T = vT_h^T qT_h per head; K=64, two heads in PE row groups.
  - exp on the Scalar engine straight out of PSUM (bf16 out).  Softmax
    max subtraction is skipped: |scores| <~ 15, within fp32 exp range.
  - attnU^T = [v_h | 1]^T E_h accumulated over Lk; row 64 is the
    denominator S.  1/S via reciprocal_approx_fast on the psum row
    (fp32, ~51 ULP), DMA-hopped to partition 0, then partition-broadcast
    (SWDGE) to 64 partitions -- no PE or big DVE copies in the
    normalize chain.  attn^T = attnU^T * r on the DVE, pipelined two
    pairs deep so nothing stalls on the DMA-latency-heavy path.
  - fc: out = tanh([attn | Q]^T-chunks^T @ Wf + bf) in natural layout,
    bf added via a partition-broadcast tile, tanh + output DMA per
    128-row chunk so only the last chunk's tail is exposed.
"""

import numpy as np

B, LQ, LK = 16, 512, 1024
F, H, D = 512, 8, 64
NCORES = 8
BPC = B // NCORES  # batches per core

_CACHE = {}


def _split_sync_waits(nc, mybir, maxw=1):
    """This container's walrus rejects instructions with more than one sync
    wait ("Too many sync wait commands").  Move excess waits onto NoOp
    instructions inserted just before the over-subscribed instruction on the
    same engine queue (program order preserves the wait semantics)."""
    for fn in nc.m.functions:
        for blk in fn.blocks:
            insts = blk.instructions
            i = 0
            while i < len(insts):
                inst = insts[i]
                si = getattr(inst, "sync_info", None)
                if si is not None and len(si.on_wait) > maxw:
                    waits = list(si.on_wait)
                    del si.on_wait[maxw:]
                    pre = []
                    for j in range(maxw, len(waits), maxw):
                        nop = mybir.InstNoOp(
                            name=nc.get_next_instruction_name(),
                            engine=inst.engine,
                            ins=[],
                            outs=[],
                            sync_info=mybir.SyncInfo(
                                on_wait=waits[j:j + maxw], on_update=[]),
                        )
                        pre.append(nop)
                    insts[i:i] = pre
                    i += len(pre)
                i += 1


def _patch_sem_clear_chunking(bass, chunk=16):
    """walrus here rejects the kernel-tail SEM_RANGE_CLEAR ISA op when the
    semaphore range is large ("ISA wrong length").  Chunk the ranges."""
    if getattr(bass.Bass.clear_and_free_semaphores, "_chunked", False):
        return
    orig = bass.Bass.clear_and_free_semaphores

    def chunked(self, sems):
        sems = list(sems)
        nums = [s.num if hasattr(s, "num") else s for s in sems]
        order = sorted(range(len(sems)), key=lambda i: nums[i])
        for j in range(0, len(sems), chunk):
            orig(self, [sems[i] for i in order[j:j + chunk]])

    chunked._chunked = True
    bass.Bass.clear_and_free_semaphores = chunked


def _build():
    import concourse.bass as bass
    import concourse.tile as tile
    from concourse import mybir

    _patch_sem_clear_chunking(bass)

    dt = mybir.dt
    f32, bf16 = dt.float32, dt.bfloat16
    AF = mybir.ActivationFunctionType
    OP = mybir.AluOpType

    nc = bass.Bass("TRN2", target_bir_lowering=False, debug=False,
                   num_devices=NCORES)

    Qd = nc.dram_tensor("Q", [BPC, LQ, F], f32, kind="ExternalInput").ap()
    Vd = nc.dram_tensor("V", [BPC, LK, F], f32, kind="ExternalInput").ap()
    Wqd = nc.dram_tensor("Wq", [F, H * D], f32, kind="ExternalInput").ap()
    bqd = nc.dram_tensor("bq", [H * D], f32, kind="ExternalInput").ap()
    Wvd = nc.dram_tensor("Wv", [F, H * D], f32, kind="ExternalInput").ap()
    bvd = nc.dram_tensor("bv", [H * D], f32, kind="ExternalInput").ap()
    Wfd = nc.dram_tensor("Wf", [F + H * D, F], f32, kind="ExternalInput").ap()
    bfd = nc.dram_tensor("bf", [F], f32, kind="ExternalInput").ap()
    Od = nc.dram_tensor("O", [BPC, LQ, F], f32, kind="ExternalOutput").ap()

    Qbf = nc.dram_tensor("Qbf", [BPC, LQ, F], bf16).ap()
    Vbf = nc.dram_tensor("Vbf", [BPC, LK, F], bf16).ap()

    with tile.TileContext(nc) as tc:
        import contextlib
        with contextlib.ExitStack() as ctx:
            def pool(name, bufs, space="SBUF"):
                return ctx.enter_context(
                    tc.tile_pool(name=name, bufs=bufs, space=space))

            const_p = pool("const", 1)
            qt_p = pool("qt", 2)        # Q^T (bf16 input transpose)
            vt_p = pool("vt", 2)        # V^T
            qproj_p = pool("qproj", 2)  # qT
            vproj_p = pool("vproj", 2)  # vT
            vn_p = pool("vn", 2)        # v natural (+ones col), from vT
            vns_p = pool("vns", 3)      # v natural staging (transpose out)
            e_p = pool("E", 4)          # exp(scores^T) per head
            at_p = pool("attnT", 2)
            au_p = pool("au", 5)
            rs_p = pool("rs", 2)        # S rows on partition 64
            s8_p = pool("s8", 2)        # S reshaped [128, 8]
            r8_p = pool("r8", 2)        # 1/S [128, 8]
            r2f_p = pool("r2f", 2)      # 1/S on partition 0
            rbc_p = pool("rbc", 2)      # 1/S broadcast to 64 partitions
            ao_p = pool("anodd", 3)
            fco_p = pool("fco", 3)
            osb_p = pool("osb", 3)

            ps_small = pool("ps_small", 4, space="PSUM")   # [128,512] 1 bank
            ps_sc = pool("ps_sc", 2, space="PSUM")         # [128,2,512] 2 banks

            # ---- T1: weights/biases on the scalar HWDGE queue (idle at
            # start; keeps them off the input-cast SWDGE queue and off the
            # sync queue that carries the input transposes) ----
            bf_row = const_p.tile([1, F], f32)
            nc.scalar.dma_start(
                bf_row[:], bfd.rearrange("(a n) -> a n", a=1))
            bq_sb = const_p.tile([128, 4], f32)
            nc.scalar.dma_start(
                bq_sb[:], bqd.rearrange("(ko p) -> p ko", p=128))
            bv_sb = const_p.tile([128, 4], f32)
            nc.scalar.dma_start(
                bv_sb[:], bvd.rearrange("(ko p) -> p ko", p=128))
            wst_p = ctx.enter_context(tc.tile_pool(name="wstage", bufs=1))
            Wq_f32 = wst_p.tile([128, 4, H * D], f32, name="wstage",
                                tag="wstage")
            nc.scalar.dma_start(
                Wq_f32[:], Wqd.rearrange("(ko p) n -> p ko n", p=128))
            Wq_sb = const_p.tile([128, 4, H * D], bf16)
            nc.vector.tensor_copy(Wq_sb[:], Wq_f32[:])
            Wv_f32 = wst_p.tile([128, 4, H * D], f32, name="wstage",
                                tag="wstage")
            nc.scalar.dma_start(
                Wv_f32[:], Wvd.rearrange("(ko p) n -> p ko n", p=128))
            Wv_sb = const_p.tile([128, 4, H * D], bf16)
            nc.vector.tensor_copy(Wv_sb[:], Wv_f32[:])

            # SWDGE queue (serial, ~150GB/s): only the input casts + the
            # late-needed Wf cast-load, in need order.  The bf broadcast
            # rides the sync HWDGE ring instead.
            nc.gpsimd.dma_start(Qbf[0], Qd[0])
            nc.gpsimd.dma_start(Vbf[0], Vd[0])
            nc.gpsimd.dma_start(Qbf[1], Qd[1])
            nc.gpsimd.dma_start(Vbf[1], Vd[1])
            Wf_sb = const_p.tile([128, 8, F], bf16)
            nc.gpsimd.dma_start(
                Wf_sb[:], Wfd.rearrange("(ko p) n -> p ko n", p=128))
            bfb = const_p.tile([128, F], f32)
            nc.sync.dma_start(
                bfb[:],
                bf_row[0:1, :].unsqueeze(1).broadcast_to((1, 128, F)))

            # ---- T2: input transposes.  Batch 0 chunked: Q chunks on the
            # scalar queue, V chunks on the sync queue (parallel dispatch).
            # Batch 1 whole-tensor on sync, issued early (mid-batch-0) so
            # they are ahead of batch 0's semaphore-waiting attn DMAs. ----
            QTs, VTs = [], []
            for b in range(BPC):
                QT = qt_p.tile([128, 4, LQ], bf16, name="QT", tag="QT")
                VT = vt_p.tile([128, 4, LK], bf16, name="VT", tag="VT")
                QTs.append(QT)
                VTs.append(VT)

            nc.scalar.dma_start(QTs[0][:, :, :], Qbf[0], transpose=True)
            nc.scalar.dma_start(VTs[0][:, :, :], Vbf[0], transpose=True)

            def issue_b1_input_transposes():
                nc.sync.dma_start(QTs[1][:, :, :], Qbf[1], transpose=True)
                nc.sync.dma_start(VTs[1][:, :, :], Vbf[1], transpose=True)

            for b in range(BPC):
                QT, VT = QTs[b], VTs[b]

                # ---- T3: projections (qT first: its inputs land first) ----
                qT = qproj_p.tile([128, 4, LQ], bf16)
                for m in range(4):
                    ps = ps_small.tile([128, 512], f32, name="ps", tag="ps")
                    for kk in range(4):
                        nc.tensor.matmul(
                            ps[:], Wq_sb[:, kk, m * 128:(m + 1) * 128],
                            QT[:, kk, :], start=(kk == 0), stop=(kk == 3))
                    nc.vector.tensor_scalar_add(
                        qT[:, m, :], ps[:], bq_sb[:, m:m + 1])

                # vT m-major so pair 0's full Lk row block finishes first;
                # v natural (+bias via vT) via DMA transposes right behind
                # each m block.
                vT = vproj_p.tile([128, 4, LK], bf16)
                vn = vn_p.tile([128, 8, 8, 68], bf16)
                nc.vector.memset(vn[:, :, :, 64:65], 1.0)
                for m in range(4):
                    for n in range(2):
                        ps = ps_small.tile([128, 512], f32, name="ps",
                                           tag="ps")
                        for kk in range(4):
                            nc.tensor.matmul(
                                ps[:], Wv_sb[:, kk, m * 128:(m + 1) * 128],
                                VT[:, kk, n * 512:(n + 1) * 512],
                                start=(kk == 0), stop=(kk == 3))
                        nc.vector.tensor_scalar_add(
                            vT[:, m, n * 512:(n + 1) * 512], ps[:],
                            bv_sb[:, m:m + 1])
                    # One-shot transpose of the m block ([128, 1024] ->
                    # logical [1024, 128] laid out [128, 8, 128]) into
                    # staging, then a DVE re-stride into the 68-padded vn.
                    vnm = vns_p.tile([128, 8, 128], bf16, name="vnm",
                                     tag="vnm")
                    nc.sync.dma_start(vnm[:], vT[:, m, :], transpose=True)
                    nc.vector.tensor_copy(
                        vn[:, :, 2 * m:2 * m + 2, 0:64],
                        vnm[:].rearrange("p c (h d) -> p c h d", d=64))

                if b == 0:
                    issue_b1_input_transposes()

                # ---- T4+T5: attention, head-pair at a time.  The
                # normalize chain (recip -> partition hop -> broadcast ->
                # scale) is pipelined at PAIR granularity: stage A right
                # after a pair's attn matmuls, B one pair later, C two
                # pairs later. ----
                attnT = at_p.tile([128, 4, LQ], bf16)
                chain = []

                def stage_B(st):
                    # 1/S for both heads in [128,8] shape (the iterative
                    # divide is free-dim-serial, so the DMA reshape makes
                    # it ~40x cheaper than on [1,1024]), then back to
                    # partition 0 for the broadcast read.  fp32 throughout.
                    st["r8"] = r8_p.tile([128, 8], f32, name="r8", tag="r8")
                    nc.vector.reciprocal(st["r8"][:], st["s8"][:])
                    st["r2f"] = r2f_p.tile([1, 2, 512], f32, name="r2f",
                                           tag="r2f")
                    nc.sync.dma_start(st["r2f"][:], st["r8"][:])

                def stage_C(st):
                    # HWDGE broadcast-read DMA of [1/S_even | 1/S_odd]
                    # to 64 partitions, both heads in one op.
                    st["rbc"] = rbc_p.tile([64, 2, 512], f32, name="rbc",
                                           tag="rbc")
                    nc.sync.dma_start(
                        st["rbc"][:],
                        st["r2f"][0:1, :, :].unsqueeze(1)
                        .broadcast_to((1, 64, 2, 512)))
                    nc.vector.tensor_tensor(
                        st["attnT"][0:64, st["p"], :], st["au"][0][:],
                        st["rbc"][:, 0, :], op=OP.mult)
                    an = ao_p.tile([64, 512], bf16, name="an", tag="an")
                    nc.vector.tensor_tensor(
                        an[:], st["au"][1][:], st["rbc"][:, 1, :],
                        op=OP.mult)
                    nc.sync.dma_start(
                        st["attnT"][64:128, st["p"], :], an[:])

                for p in range(4):
                    E2 = [e_p.tile([128, 8, 512], bf16, name=f"E{i}",
                                   tag="E") for i in range(2)]
                    for g in range(4):
                        psA = ps_sc.tile([128, 2, 512], f32, name="psA",
                                         tag="sc")
                        psB = ps_sc.tile([128, 2, 512], f32, name="psB",
                                         tag="sc")
                        for i in range(2):
                            c = 2 * g + i
                            nc.tensor.matmul(
                                psA[:, i, :],
                                vT[0:64, p, c * 128:(c + 1) * 128],
                                qT[0:64, p, :], start=True, stop=True)
                            nc.tensor.matmul(
                                psB[:, i, :],
                                vT[64:128, p, c * 128:(c + 1) * 128],
                                qT[64:128, p, :], start=True, stop=True)
                        nc.scalar.activation(
                            E2[0][:, 2 * g:2 * g + 2, :], psA[:], AF.Exp)
                        nc.scalar.activation(
                            E2[1][:, 2 * g:2 * g + 2, :], psB[:], AF.Exp)

                    st = {"p": p, "attnT": attnT, "au": [None, None]}
                    rs = rs_p.tile([65, 2, 512], f32, name="rs", tag="rs")
                    for s in range(2):
                        h = 2 * p + s
                        E = E2[s]
                        psAt = ps_small.tile([128, 512], f32, name="psAt",
                                             tag="ps")
                        for c in range(8):
                            nc.tensor.matmul(
                                psAt[0:65, :], vn[:, c, h, 0:65], E[:, c, :],
                                start=(c == 0), stop=(c == 7))
                        au = au_p.tile([64, 512], f32, name="au", tag="au")
                        nc.vector.tensor_copy(au[:], psAt[0:64, :])
                        st["au"][s] = au
                        nc.vector.tensor_copy(rs[64:65, s, :],
                                              psAt[64:65, :])
                    st["s8"] = s8_p.tile([128, 8], f32, name="s8", tag="s8")
                    nc.sync.dma_start(st["s8"][:], rs[64:65, :, :])
                    chain.append(st)
                    if len(chain) >= 2:
                        stage_B(chain[-2])
                    if len(chain) >= 3:
                        stage_C(chain[-3])

                # ---- T6: fc + tanh.  Q-half first (no attnT dep) so the
                # PE has work while the last pairs' normalize drains;
                # per-chunk drain/tanh/output so only chunk 3's tail is
                # exposed. ----
                stage_B(chain[-1])
                psOs = []
                for m in range(4):
                    psO = ps_small.tile([128, 512], f32, name="psO",
                                        tag="ps")
                    for kk in (4, 5, 6, 7):
                        nc.tensor.matmul(
                            psO[:], QT[:, kk - 4, m * 128:(m + 1) * 128],
                            Wf_sb[:, kk, :], start=(kk == 4), stop=False)
                    psOs.append(psO)
                stage_C(chain[-2])
                stage_C(chain[-1])
                chain.clear()
                for m in range(4):
                    psO = psOs[m]
                    for kk in range(4):
                        nc.tensor.matmul(
                            psO[:], attnT[:, kk, m * 128:(m + 1) * 128],
                            Wf_sb[:, kk, :], start=False, stop=(kk == 3))
                    fco = fco_p.tile([128, F], f32, name="fco", tag="fco")
                    nc.vector.tensor_tensor(
                        fco[:], psO[:], bfb[:], op=OP.add)
                    osb = osb_p.tile([128, F], f32, name="osb", tag="osb")
                    nc.scalar.activation(osb[:], fco[:], AF.Tanh)
                    nc.scalar.dma_start(
                        Od[b][m * 128:(m + 1) * 128, :], osb[:])

    _split_sync_waits(nc, mybir)
    return nc


def _get_nc():
    if "nc" not in _CACHE:
        _CACHE["nc"] = _build()
    return _CACHE["nc"]


def kernel(Q, V, Wq, bq, Wv, bv, Wf, bf, _trace=False):
    from concourse.bass_utils import run_bass_kernel_spmd

    nc = _get_nc()
    Q = np.ascontiguousarray(np.asarray(Q, dtype=np.float32))
    V = np.ascontiguousarray(np.asarray(V, dtype=np.float32))
    shared = {
        "Wq": np.ascontiguousarray(np.asarray(Wq, np.float32)),
        "bq": np.ascontiguousarray(np.asarray(bq, np.float32)),
        "Wv": np.ascontiguousarray(np.asarray(Wv, np.float32)),
        "bv": np.ascontiguousarray(np.asarray(bv, np.float32)),
        "Wf": np.ascontiguousarray(np.asarray(Wf, np.float32)),
        "bf": np.ascontiguousarray(np.asarray(bf, np.float32)),
    }
    in_maps = []
    for c in range(NCORES):
        m = {"Q": Q[c * BPC:(c + 1) * BPC], "V": V[c * BPC:(c + 1) * BPC]}
        m.update(shared)
        in_maps.append(m)

    res = run_bass_kernel_spmd(nc, in_maps, core_ids=list(range(NCORES)),
                               trace=_trace)
    out = np.concatenate([res.results[c]["O"] for c in range(NCORES)], axis=0)
    if _trace:
        _CACHE["last_exec_time_ns"] = res.exec_time_ns
    return out
